# revision 4
# baseline (speedup 1.0000x reference)
"""TRN2 Bass kernel for nn_DistributionalQNetwork (C51 categorical projection).

Contract: kernel(**inputs) takes FULL unsharded numpy inputs (B=131072) and
returns the FULL [B, 251] projected distribution, matching reference.py.

Strategy (8-way batch data parallel, 16384 rows/core):
  - MLP on the PE in float32r (TF32-like, full rate at N>=512), activations
    kept transposed [feature, row] so the contraction dim stays on partitions.
  - leaky-relu + bias fused into ACT passes straight out of PSUM.
  - logits transposed back per 128-row chunk via PE; exp on ACT (with row-sum
    accumulation); softmax normalization deferred to the final scale.
  - C51 projection per 128-row tile on the DVE: the Bellman-backed support
    b = clip(r + g*z) mapped to bins; keys are sorted with steps in {0,1}, so
    per-bin sums are differences of a prefix scan at run-end positions;
    run-end values are scattered to their bins with GPSIMD local_scatter
    (fp32 split into int16 halves, even/odd interleaved).
  - The reference's b uses a true fp32 division by delta_z; DVE has no divide,
    so b is computed as s*1.25 (<=1ulp high) plus an exact Sterbenz-split
    boundary test (DELTA = s - m*dz) to recover the reference's exact-integer
    (l==u) semantics bit-for-bit.
"""

import sys

sys.path.insert(0, "/opt/trn_rl_repo")
sys.path.insert(0, "/opt/pypackages")

from contextlib import ExitStack

import numpy as np

import concourse.bass as bass
import concourse.tile as tile
from concourse import bacc, mybir
from concourse.masks import make_identity

F32 = mybir.dt.float32
F32R = mybir.dt.float32r
I16 = mybir.dt.int16
I32 = mybir.dt.int32
AL = mybir.AluOpType
AF = mybir.ActivationFunctionType

NCORES = 8
B_FULL = 131072
ROWS = B_FULL // NCORES  # 16384 rows per core
OBS, ACTD, IN, H, NA = 96, 32, 128, 512, 251
NB = 260  # fp32 columns of scatter dst (1 zero pad + 256 bins + 3 spare)
NE = 2 * NB  # int16 elements of scatter dst

# exact split of dz = fl(0.8) = 13421773 * 2^-24 for the boundary test
DZI = 13421773
DZH = float((DZI >> 12) * 2.0**-12)
DZL = float((DZI & 0xFFF) * 2.0**-24)
THRC = float(DZI * 2.0**-48)  # dz * 2^-24 (exact fp32)
P23 = float(2**23)


def _build_kernel(ctx, tc, aps, rows):
    nc = tc.nc
    n_tiles = rows // 128
    n_blocks = rows // 512

    const = ctx.enter_context(tc.tile_pool(name="const", bufs=1))
    scal = ctx.enter_context(tc.tile_pool(name="scal", bufs=1))
    xpool = ctx.enter_context(tc.tile_pool(name="xpool", bufs=3))
    htile = ctx.enter_context(tc.tile_pool(name="htile", bufs=2))
    ltile = ctx.enter_context(tc.tile_pool(name="ltile", bufs=4))
    expp = ctx.enter_context(tc.tile_pool(name="expp", bufs=8))
    work = ctx.enter_context(tc.tile_pool(name="work", bufs=2))
    outp = ctx.enter_context(tc.tile_pool(name="outp", bufs=4))
    ps_mm = ctx.enter_context(tc.tile_pool(name="ps_mm", bufs=3, space="PSUM"))
    ps_l = ctx.enter_context(tc.tile_pool(name="ps_l", bufs=2, space="PSUM"))
    ps_tr = ctx.enter_context(tc.tile_pool(name="ps_tr", bufs=3, space="PSUM"))

    # ---------------- constants ----------------
    ident = const.tile([128, 128], F32)
    make_identity(nc, ident[:])

    zrep = const.tile([128, NA], F32)
    nc.sync.dma_start(zrep[:], aps["q_support"][None, :].to_broadcast([128, NA]))

    emask = const.tile([128, 1], I32)
    nc.vector.memset(emask[:], 0x7F800000)
    smask = const.tile([128, 1], I32)
    nc.vector.memset(smask[:], 0x7FFFFFFF)

    # weights -> float32r
    def load_f32r(name, shape):
        t0 = const.tile(shape, F32, tag=f"{name}_raw")
        nc.sync.dma_start(t0[:], aps[name][:])
        t1 = const.tile(shape, F32R, tag=f"{name}_r")
        nc.vector.tensor_copy(t1[:], t0[:])
        return t1

    w1 = load_f32r("W1", [128, H])
    w2 = [None] * 4
    w3 = [None] * 4
    for k in range(4):
        t0 = const.tile([128, H], F32, tag=f"w2raw{k}")
        nc.sync.dma_start(t0[:], aps["W2"][128 * k : 128 * (k + 1), :])
        w2[k] = const.tile([128, H], F32R, name=f"w2r{k}", tag=f"w2r{k}")
        nc.vector.tensor_copy(w2[k][:], t0[:])
        t3 = const.tile([128, NA], F32, tag=f"w3raw{k}")
        nc.sync.dma_start(t3[:], aps["W3"][128 * k : 128 * (k + 1), :])
        w3[k] = const.tile([128, NA], F32R, name=f"w3r{k}", tag=f"w3r{k}")
        nc.vector.tensor_copy(w3[k][:], t3[:])

    b1t = const.tile([128, 4], F32)
    nc.sync.dma_start(b1t[:], aps["b1"].rearrange("(m p) -> p m", p=128))
    b2t = const.tile([128, 4], F32)
    nc.sync.dma_start(b2t[:], aps["b2"].rearrange("(m p) -> p m", p=128))
    b3a = const.tile([128, 1], F32)
    nc.sync.dma_start(b3a[:], aps["b3"][0:128][:, None])
    b3b = const.tile([123, 1], F32)
    nc.sync.dma_start(b3b[:], aps["b3"][128:251][:, None])

    # per-row scalars: [128, n_tiles] with column t = rows [128t, 128t+128)
    rew = scal.tile([128, n_tiles], F32)
    nc.sync.dma_start(rew[:], aps["rewards"].rearrange("(t p) -> p t", p=128))
    boo = scal.tile([128, n_tiles], F32)
    nc.sync.dma_start(boo[:], aps["bootstrap"].rearrange("(t p) -> p t", p=128))
    dis = scal.tile([128, n_tiles], F32)
    nc.sync.dma_start(dis[:], aps["discount"].rearrange("(t p) -> p t", p=128))
    gsc = scal.tile([128, n_tiles], F32)
    nc.vector.tensor_tensor(out=gsc[:], in0=boo[:], in1=dis[:], op=AL.mult)

    obs_v = aps["obs"]
    act_v = aps["actions"]
    out_v = aps["proj"]

    def phase_b(b):
        """MLP for rows [512b, 512b+512) -> EXPR tiles + SE per row-chunk."""
        r0 = 512 * b
        # load + transpose X
        xt = xpool.tile([128, 512], F32R, tag="xt")
        for c in range(4):
            xc = xpool.tile([128, 128], F32, tag="xc")
            nc.sync.dma_start(xc[:, 0:OBS], obs_v[r0 + 128 * c : r0 + 128 * (c + 1), :])
            nc.sync.dma_start(
                xc[:, OBS:IN], act_v[r0 + 128 * c : r0 + 128 * (c + 1), :]
            )
            tp = ps_tr.tile([128, 128], F32, space="PSUM", tag="pstr", name="tp_x")
            nc.tensor.transpose(tp[:], xc[:], ident[:])
            nc.vector.tensor_copy(xt[:, 128 * c : 128 * (c + 1)], tp[:])

        # layer 1 + 2
        h1 = [None] * 4
        for m in range(4):
            ps = ps_mm.tile([128, 512], F32, space="PSUM", tag="psmm", name="ps1")
            nc.tensor.matmul(
                ps[:], lhsT=w1[:, 128 * m : 128 * (m + 1)], rhs=xt[:],
                start=True, stop=True,
            )
            h1[m] = htile.tile([128, 512], F32R, name=f"h1_{m}", tag=f"h1_{m}")
            nc.scalar.activation(
                h1[m][:], ps[:], AF.Lrelu, bias=b1t[:, m : m + 1], scale=1.0,
                alpha=0.01,
            )
        h2 = [None] * 4
        for m in range(4):
            ps = ps_mm.tile([128, 512], F32, space="PSUM", tag="psmm", name="ps2")
            for k in range(4):
                nc.tensor.matmul(
                    ps[:], lhsT=w2[k][:, 128 * m : 128 * (m + 1)], rhs=h1[k][:],
                    start=(k == 0), stop=(k == 3),
                )
            h2[m] = htile.tile([128, 512], F32R, name=f"h2_{m}", tag=f"h2_{m}")
            nc.scalar.activation(
                h2[m][:], ps[:], AF.Lrelu, bias=b2t[:, m : m + 1], scale=1.0,
                alpha=0.01,
            )
        # layer 3 -> logitsT (+bias) in fp32 SBUF
        lt0 = ltile.tile([128, 512], F32, tag="lt0")
        lt1 = ltile.tile([123, 512], F32, tag="lt1")
        for m, (lt, bb, w) in enumerate(((lt0, b3a, 128), (lt1, b3b, 123))):
            ps = ps_l.tile([128, 512], F32, space="PSUM", tag="psL", name=f"psL{m}")[0:w, :]
            for k in range(4):
                nc.tensor.matmul(
                    ps[:], lhsT=w3[k][:, 128 * m : 128 * m + w], rhs=h2[k][:],
                    start=(k == 0), stop=(k == 3),
                )
            nc.scalar.activation(lt[:], ps[:], AF.Identity, bias=bb[:], scale=1.0)

        # per row-chunk: transpose logits back, exp with row-sum accumulation
        res = []
        for c in range(4):
            t0 = ps_tr.tile([128, 128], F32, space="PSUM", tag="pstr", name="tL0")
            nc.tensor.transpose(t0[:], lt0[:, 128 * c : 128 * (c + 1)], ident[:])
            t1 = ps_tr.tile([128, 128], F32, space="PSUM", tag="pstr", name="tL1")[:, 0:123]
            nc.tensor.transpose(
                t1[:], lt1[:, 128 * c : 128 * (c + 1)], ident[0:123, 0:123]
            )
            ex = expp.tile([128, NA], F32, tag="ex")
            se = expp.tile([128, 2], F32, tag="se")
            nc.scalar.activation(
                ex[:, 0:128], t0[:], AF.Exp, bias=0.0, scale=1.0,
                accum_out=se[:, 0:1],
            )
            nc.scalar.activation(
                ex[:, 128:NA], t1[:], AF.Exp, bias=0.0, scale=1.0,
                accum_out=se[:, 1:2],
            )
            res.append((ex, se))
        return res

    def phase_c(t, ex, se):
        """C51 projection for row-tile t (rows [128t, 128t+128))."""
        g_sc = gsc[:, t : t + 1]
        r_sc = rew[:, t : t + 1]

        # b pipeline (rounding-exact vs reference)
        t1 = work.tile([128, NA], F32, tag="t1")
        nc.vector.tensor_scalar(out=t1[:], in0=zrep[:], scalar1=g_sc, scalar2=None,
                                op0=AL.mult)
        t2 = work.tile([128, NA], F32, tag="t2")
        nc.vector.tensor_scalar(out=t2[:], in0=t1[:], scalar1=r_sc, scalar2=100.0,
                                op0=AL.add, op1=AL.add)
        s = work.tile([128, NA], F32, tag="s")
        nc.vector.tensor_scalar(out=s[:], in0=t2[:], scalar1=0.0, scalar2=200.0,
                                op0=AL.max, op1=AL.min)
        b0 = work.tile([128, NA], F32, tag="b0")
        nc.scalar.activation(b0[:], s[:], AF.Copy, bias=0.0, scale=1.25)

        # m = rn(b0); exact boundary test for the reference's eq cases
        mm = work.tile([128, NA], F32, tag="mm")
        nc.vector.tensor_scalar(out=mm[:], in0=b0[:], scalar1=P23, scalar2=P23,
                                op0=AL.add, op1=AL.subtract)
        d1 = work.tile([128, NA], F32, tag="d1")
        nc.vector.scalar_tensor_tensor(out=d1[:], in0=mm[:], scalar=-DZH,
                                       in1=s[:], op0=AL.mult, op1=AL.add)
        delta = work.tile([128, NA], F32, tag="delta")
        nc.vector.scalar_tensor_tensor(out=delta[:], in0=mm[:], scalar=-DZL,
                                       in1=d1[:], op0=AL.mult, op1=AL.add)
        pw = work.tile([128, NA], I32, tag="pw")
        nc.vector.tensor_tensor(out=pw[:], in0=mm[:].bitcast(I32),
                                in1=emask[:].to_broadcast([128, NA]),
                                op=AL.bitwise_and)
        absd = work.tile([128, NA], F32, tag="absd")
        nc.vector.tensor_tensor(out=absd[:].bitcast(I32),
                                in0=delta[:].bitcast(I32),
                                in1=smask[:].to_broadcast([128, NA]),
                                op=AL.bitwise_and)
        eqd = work.tile([128, NA], F32, tag="eqd")
        nc.vector.scalar_tensor_tensor(out=eqd[:], in0=pw[:].bitcast(F32),
                                       scalar=THRC, in1=absd[:], op0=AL.mult,
                                       op1=AL.is_gt)
        eqm = work.tile([128, NA], F32, tag="eqm")
        nc.vector.scalar_tensor_tensor(out=eqm[:], in0=s[:], scalar=0.0,
                                       in1=eqd[:], op0=AL.is_equal, op1=AL.max)

        # floor(b0) from rn: lf = mm - (mm > b0)
        cmp = work.tile([128, NA], F32, tag="cmp")
        nc.vector.tensor_tensor(out=cmp[:], in0=mm[:], in1=b0[:], op=AL.is_gt)
        lf = work.tile([128, NA], F32, tag="lf")
        nc.vector.tensor_tensor(out=lf[:], in0=mm[:], in1=cmp[:], op=AL.subtract)

        frac = work.tile([128, NA], F32, tag="frac")
        nc.vector.tensor_tensor(out=frac[:], in0=b0[:], in1=lf[:], op=AL.subtract)
        e1 = work.tile([128, NA], F32, tag="e1")
        nc.vector.scalar_tensor_tensor(out=e1[:], in0=lf[:], scalar=1.0,
                                       in1=eqm[:], op0=AL.is_ge, op1=AL.mult)
        e2 = work.tile([128, NA], F32, tag="e2")
        nc.vector.scalar_tensor_tensor(out=e2[:], in0=lf[:], scalar=249.0,
                                       in1=eqm[:], op0=AL.is_le, op1=AL.mult)

        # weights (with exp values; softmax norm deferred)
        s1 = work.tile([128, NA], F32, tag="s1")
        nc.vector.tensor_tensor(out=s1[:], in0=frac[:], in1=eqm[:], op=AL.add)
        s2 = work.tile([128, NA], F32, tag="s2")
        nc.vector.scalar_tensor_tensor(out=s2[:], in0=s1[:], scalar=-1.0,
                                       in1=e2[:], op0=AL.mult, op1=AL.add)
        wl = work.tile([128, NA], F32, tag="wl")
        nc.vector.scalar_tensor_tensor(out=wl[:], in0=s2[:], scalar=1.0,
                                       in1=ex[:], op0=AL.add, op1=AL.mult)
        s4 = work.tile([128, NA], F32, tag="s4")
        nc.vector.tensor_tensor(out=s4[:], in0=frac[:], in1=e1[:], op=AL.add)
        wu = work.tile([128, NA], F32, tag="wu")
        nc.vector.tensor_tensor(out=wu[:], in0=s4[:], in1=ex[:], op=AL.mult)

        # keys
        l_fin = work.tile([128, NA], F32, tag="l_fin")
        nc.vector.tensor_tensor(out=l_fin[:], in0=lf[:], in1=e1[:], op=AL.subtract)
        v1 = work.tile([128, NA], F32, tag="v1")
        nc.vector.scalar_tensor_tensor(out=v1[:], in0=eqm[:], scalar=-1.0,
                                       in1=e2[:], op0=AL.mult, op1=AL.add)
        u_fin = work.tile([128, NA], F32, tag="u_fin")
        nc.vector.scalar_tensor_tensor(out=u_fin[:], in0=v1[:], scalar=1.0,
                                       in1=lf[:], op0=AL.add, op1=AL.add)

        # prefix scans
        zeros = work.tile([128, NA], F32, tag="zeros")
        nc.vector.memset(zeros[:], 0.0)
        cl = work.tile([128, NA], F32, tag="cl")
        nc.vector.tensor_tensor_scan(out=cl[:], data0=wl[:], data1=zeros[:],
                                     initial=0.0, op0=AL.add, op1=AL.add)
        cu = work.tile([128, NA], F32, tag="cu")
        nc.vector.tensor_tensor_scan(out=cu[:], data0=wu[:], data1=zeros[:],
                                     initial=0.0, op0=AL.add, op1=AL.add)

        # scatter run-end scan values into bins (int16-pair trick)
        def scatter_side(keys, cvals, dn):
            rend = work.tile([128, NA], F32, tag=f"rend{dn}")
            nc.vector.tensor_tensor(out=rend[:, 0 : NA - 1], in0=keys[:, 0 : NA - 1],
                                    in1=keys[:, 1:NA], op=AL.not_equal)
            nc.vector.memset(rend[:, NA - 1 : NA], 1.0)
            pre = work.tile([128, NA], F32, tag=f"pre{dn}")
            nc.vector.tensor_scalar(out=pre[:], in0=keys[:], scalar1=2.0,
                                    scalar2=4.0, op0=AL.mult, op1=AL.add)
            idxf0 = work.tile([128, NA], F32, tag=f"idxf0{dn}")
            nc.vector.tensor_tensor(out=idxf0[:], in0=pre[:], in1=rend[:],
                                    op=AL.mult)
            idxI = work.tile([128, 2 * NA], I16, tag=f"idxI{dn}")
            iv = idxI[:].rearrange("p (n two) -> p n two", two=2)
            nc.vector.tensor_scalar(out=iv[:, :, 0], in0=idxf0[:], scalar1=2.0,
                                    scalar2=None, op0=AL.subtract)
            nc.vector.tensor_scalar(out=iv[:, :, 1], in0=idxf0[:], scalar1=1.0,
                                    scalar2=None, op0=AL.subtract)
            dst = work.tile([128, NE], I16, tag=f"dst{dn}")
            nc.gpsimd.local_scatter(
                out_ap=dst[:], data_ap=cvals[:].bitcast(I16), idxs_ap=idxI[:],
                channels=128, num_elems=NE, num_idxs=2 * NA,
            )
            return dst

        dstl = scatter_side(l_fin, cl, "l")
        dstu = scatter_side(u_fin, cu, "u")

        # assemble: proj = (relu(dDL) + relu(dDU)) / sum(exp)
        sesum = work.tile([128, 1], F32, tag="sesum")
        nc.vector.tensor_tensor(out=sesum[:], in0=se[:, 0:1], in1=se[:, 1:2],
                                op=AL.add)
        recip = work.tile([128, 1], F32, tag="recip")
        nc.vector.reciprocal(recip[:], sesum[:])

        DL = dstl[:].bitcast(F32)
        DU = dstu[:].bitcast(F32)
        dl_ = work.tile([128, NA], F32, tag="dl_")
        nc.vector.tensor_tensor(out=dl_[:], in0=DL[:, 1 : NA + 1], in1=DL[:, 0:NA],
                                op=AL.subtract)
        pl = work.tile([128, NA], F32, tag="pl")
        nc.vector.tensor_scalar(out=pl[:], in0=dl_[:], scalar1=0.0, scalar2=None,
                                op0=AL.max)
        du_ = work.tile([128, NA], F32, tag="du_")
        nc.vector.tensor_tensor(out=du_[:], in0=DU[:, 1 : NA + 1], in1=DU[:, 0:NA],
                                op=AL.subtract)
        pall = outp.tile([128, NA], F32, tag="pall")
        nc.vector.scalar_tensor_tensor(out=pall[:], in0=du_[:], scalar=0.0,
                                       in1=pl[:], op0=AL.max, op1=AL.add)
        proj = outp.tile([128, NA], F32, tag="proj")
        nc.scalar.activation(proj[:], pall[:], AF.Copy, bias=0.0, scale=recip[:])
        nc.sync.dma_start(out_v[128 * t : 128 * (t + 1), :], proj[:])

    for b in range(n_blocks):
        res = phase_b(b)
        for c in range(4):
            ex, se = res[c]
            phase_c(4 * b + c, ex, se)


def build_program(rows=ROWS, num_devices=NCORES):
    nc = bacc.Bacc(
        "TRN2",
        target_bir_lowering=False,
        debug=False,
        enable_asserts=True,
        num_devices=num_devices,
    )
    aps = {}
    specs = {
        "obs": [rows, OBS],
        "actions": [rows, ACTD],
        "rewards": [rows],
        "bootstrap": [rows],
        "discount": [rows],
        "q_support": [NA],
        "W1": [IN, H],
        "b1": [H],
        "W2": [H, H],
        "b2": [H],
        "W3": [H, NA],
        "b3": [NA],
    }
    for name, shape in specs.items():
        aps[name] = nc.dram_tensor(name, shape, F32, kind="ExternalInput").ap()
    aps["proj"] = nc.dram_tensor("proj", [rows, NA], F32, kind="ExternalOutput").ap()

    with tile.TileContext(nc) as tc, ExitStack() as ctx:
        _build_kernel(ctx, tc, aps, rows)
    nc.compile()
    return nc


_NC_CACHE = {}


def kernel(**inputs):
    obs = np.ascontiguousarray(np.asarray(inputs["obs"], dtype=np.float32))
    B = obs.shape[0]
    rows = B // NCORES
    key = rows
    if key not in _NC_CACHE:
        _NC_CACHE[key] = build_program(rows=rows)
    nc = _NC_CACHE[key]

    full = {
        k: np.ascontiguousarray(np.asarray(inputs[k], dtype=np.float32))
        for k in (
            "obs", "actions", "rewards", "bootstrap", "discount",
            "q_support", "W1", "b1", "W2", "b2", "W3", "b3",
        )
    }
    shared = ("q_support", "W1", "b1", "W2", "b2", "W3", "b3")
    in_maps = []
    for i in range(NCORES):
        m = {}
        for k in ("obs", "actions", "rewards", "bootstrap", "discount"):
            m[k] = full[k][i * rows : (i + 1) * rows]
        for k in shared:
            m[k] = full[k]
        in_maps.append(m)

    from concourse.bass_utils import run_bass_kernel_spmd

    res = run_bass_kernel_spmd(nc, in_maps, core_ids=list(range(NCORES)))
    out = np.concatenate([res.results[i]["proj"] for i in range(NCORES)], axis=0)
    return out.astype(np.float32)


if __name__ == "__main__":
    # smoke run with random data at full size
    rng = np.random.default_rng(0)
    B = B_FULL
    inputs = dict(
        obs=rng.standard_normal((B, OBS)).astype(np.float32),
        actions=rng.standard_normal((B, ACTD)).astype(np.float32),
        rewards=rng.standard_normal(B).astype(np.float32),
        bootstrap=rng.random(B).astype(np.float32),
        discount=(0.95 + 0.05 * rng.random(B)).astype(np.float32),
        q_support=np.linspace(-100, 100, NA).astype(np.float32),
        W1=(rng.standard_normal((IN, H)) / np.sqrt(IN)).astype(np.float32),
        b1=np.zeros(H, np.float32),
        W2=(rng.standard_normal((H, H)) / np.sqrt(H)).astype(np.float32),
        b2=np.zeros(H, np.float32),
        W3=(rng.standard_normal((H, NA)) / np.sqrt(H)).astype(np.float32),
        b3=np.zeros(NA, np.float32),
    )
    out = kernel(**inputs)
    print("out", out.shape, out.dtype, out[0, :5], out.sum())
